# revision 14
# baseline (speedup 1.0000x reference)
"""Trainium2 kernel for the NNUE-style factorized embedding segment-sum.

Strategy ("flip"): the ragged two-table embedding-bag is reformulated as
block matmuls with the TABLE STATIONARY and the per-bag count columns as the
moving operand.  For output row (bag) with king-square block t, the
gather+segment-sum over its ragged feature ids equals
merged_table[t] slices.T @ count_cols, computed as 6 contraction chunks
(768 = 6*128 features) x 2 dout halves accumulating in
PSUM[dout_half=128, items].

Wins over the chunked lhsT=counts formulation (42.7us baseline):
 - items stream as matmul COLUMNS, so per-(core,slot) capacity is the max
   block size rounded to 8, not to 128: ~2% padding instead of ~12.5%, and
   the PE does 12 cycles/item (the dense floor: 768*256/128^2) -> ~21us.
 - outputs ride to HBM as uint8 (tables pre-scaled by 255, PSUM clipped to
   [0, 255] on the drain, host divides back): out traffic halves, keeping
   total chip HBM traffic ~59MB, inside the ~2.9TB/s chip ceiling for the
   ~22us window.  Counts are fp8e4 (ints <= 15 exact; larger are clipped
   and those rows recomputed on host - the Poisson(0.04) counts max at 3).
 - dummy matmuls from t~0 keep the PE busy through the DMA fill so the HAM
   clock gate (1.2 -> 2.4 GHz after one fully-busy ~3.4us window) flips
   early in the stream instead of ~8us in.

Host (integer work only): merge factor tables (tiles+(pieces+ranks+files)
*mask -> [64,768,256] fp16), build per-bag count rows in merged-table space
(output b columns flip-remapped so ONE table serves both outputs), sort the
64 blocks by descending item count and deal rank r -> (slot r//8, core r%8)
so each slot's shared SPMD capacity is tight, unscramble outputs.

Device per slot: load table tile (ACT ring) + count cols (SP ring, issued
two slots ahead, paired for line rate), 24 matmuls (2 item groups x 2 dout
halves x 6 feature chunks), clipped PSUM->u8 drains split DVE/Pool, one
batched store per slot on the ACT ring (final slot split so the
exit-gating HBM write receipt covers a small transfer).
"""

import numpy as np
import ml_dtypes

import concourse.bass as bass
import concourse.tile as tile
from concourse import bacc, mybir
from concourse.bass_utils import run_bass_kernel_spmd

N_CORES = 8
B = 16384          # bags
KPL = 12           # piece planes
DOUT = 256
PIECE = 768        # KPL * 64
NBLK = 8           # table blocks (slots) per core: 64 king squares / 8 cores
TABW = 12 * 128    # table tile cols per slot: (2 dout halves x 6 fchunks) x 128
OSCALE = 255.0     # uint8 output quantization scale (folded into the table)

# ---------------------------------------------------------------------------
# host-side integer prep tables
_sq = np.arange(64)
_PERM = (7 - _sq // 8) * 8 + _sq % 8          # vertical king-square flip
_v = np.arange(PIECE)
_vk, _vr, _vf = _v // 64, (_v % 64) // 8, _v % 8
_FLIP_COL = ((_vk + 6) % 12) * 64 + (7 - _vr) * 8 + _vf

_prog_cache = {}

NDUMMY = 18        # PE warmup matmuls riding the DMA fill window


def _groups(cap):
    g0 = (cap // 2 + 7) // 8 * 8
    return [(0, g0), (g0, cap - g0)]


def _build_program(caps: tuple):
    """Bass program for one core (SPMD across all 8).

    caps[s] = item capacity of slot s (multiple of 8, shared by all cores).
    """
    nc = bacc.Bacc("TRN2", target_bir_lowering=False, debug=False)
    f32 = mybir.dt.float32
    fp16 = mybir.dt.float16
    fp8 = mybir.dt.float8e4
    u8 = mybir.dt.uint8
    A = mybir.AluOpType

    sum_caps = sum(caps)
    cm_w = [6 * c for c in caps]               # count cols per slot
    base = np.concatenate([[0], np.cumsum(caps)]).astype(int)
    cbase = np.concatenate([[0], np.cumsum(cm_w)]).astype(int)
    maxcap = max(caps)

    tab = nc.dram_tensor("tab", [128, NBLK * TABW], fp16,
                         kind="ExternalInput").ap()
    # cm[p, cbase(s) + goff2 + j*gn + m] = count of slot-s group-g item m at
    # feature j*128+p (group-major within the slot so the slot-0 load can be
    # split at the group boundary)
    cm = nc.dram_tensor("cm", [128, int(cbase[-1])], fp8,
                        kind="ExternalInput").ap()
    # out[p, 2*base(s) + h*cap + m] = round(255*clip(result))[item m, h*128+p]
    out = nc.dram_tensor("out", [128, 2 * sum_caps], u8,
                         kind="ExternalOutput").ap()

    with tile.TileContext(nc) as tc:
        with (
            tc.tile_pool(name="tabp", bufs=5) as tabp,
            tc.tile_pool(name="cmp", bufs=4) as cmp_,
            tc.tile_pool(name="outp", bufs=3) as outp,
            tc.tile_pool(name="warmp", bufs=1) as wmp,
            tc.tile_pool(name="ps", bufs=8, space="PSUM") as psp,
        ):
            # ---- PE warmup: HAM clock gate needs ~3.4us of sustained busy
            # time to lift 1.2 -> 2.4 GHz; burn the DMA fill window on dummy
            # matmuls so the real stream runs warm.  Memsets ride DVE (idle
            # at start - GpSimd's queue is busy with framework memsets).
            wl = wmp.tile([128, 128], fp16, tag="warml")
            wr = wmp.tile([128, 128], fp16, tag="warmr")
            nc.vector.memset(wl[:], 0)
            nc.vector.memset(wr[:], 0)
            wp = psp.tile([128, 512], f32, tag="ps")
            for _ in range(NDUMMY):
                nc.tensor.matmul(wp[:, :128], lhsT=wl[:], rhs=wr[:])

            # ---- DMA plan: tables on the ACT HWDGE ring (stores come later
            # on the same ring, behind all table loads), counts on the SP
            # ring.  Slot 0's table is split in dout halves and its counts at
            # the group boundary so the first matmuls wait on less data;
            # later slots load in pairs (bigger DMAs run closer to line
            # rate, half the ring issue slots).
            tts = {}

            def load_tab(s0, nslots=1, split=1):
                tt = tabp.tile([128, nslots * TABW], fp16, tag="tab")
                w = nslots * TABW
                bnds = [w * k // split // 128 * 128 for k in range(split + 1)]
                for k in range(split):
                    nc.scalar.dma_start(
                        tt[:, bnds[k]:bnds[k + 1]],
                        tab[:, s0 * TABW + bnds[k]:s0 * TABW + bnds[k + 1]])
                for q in range(nslots):
                    tts[s0 + q] = (tt, q * TABW)

            cms = {}

            def load_cm(s0, nslots=1, bnds=None):
                w = int(cbase[s0 + nslots] - cbase[s0])
                ct = cmp_.tile([128, w], fp8, tag="cm")
                if bnds is None:
                    bnds = [0, w]
                for k in range(len(bnds) - 1):
                    nc.sync.dma_start(
                        ct[:, bnds[k]:bnds[k + 1]],
                        cm[:, int(cbase[s0]) + bnds[k]:
                           int(cbase[s0]) + bnds[k + 1]])
                off = 0
                for q in range(nslots):
                    cms[s0 + q] = (ct, off)
                    off += cm_w[s0 + q]

            # fill-critical first loads, split fine so the first receipts
            # land early: the matmul stream consumes tab0 tile-by-tile and
            # cm0 j-block by j-block within group 0
            g0n = _groups(caps[0])[0][1]
            load_tab(0, split=4)
            load_cm(0, bnds=[0, g0n, 3 * g0n, 6 * g0n, cm_w[0]])
            load_tab(1)
            load_cm(1)

            for s in range(NBLK):
                cap = caps[s]
                # keep loads two slots ahead, paired
                ns = max(tts.keys()) + 1
                while ns < NBLK and ns <= s + 3:
                    k = min(2, NBLK - ns)
                    load_tab(ns, nslots=k)
                    load_cm(ns, nslots=k)
                    ns += k

                tt, toff = tts[s]
                ct, coff = cms[s]
                outt = outp.tile([128, 2 * maxcap], u8, tag="out")
                di = 0
                for h in (0, 1):
                    goff2 = 0
                    for (goff, gn) in _groups(cap):
                        ps = psp.tile([128, 512], f32, tag="ps")
                        for j in range(6):
                            nc.tensor.matmul(
                                ps[:, :gn],
                                lhsT=tt[:, toff + (h * 6 + j) * 128:
                                        toff + (h * 6 + j + 1) * 128],
                                rhs=ct[:, coff + goff2 + j * gn:
                                       coff + goff2 + (j + 1) * gn],
                                start=(j == 0),
                                stop=(j == 5),
                            )
                        # clip(psum, 0, 255) -> u8 out tile (table carries
                        # the 255x scale; host divides back)
                        dsl = outt[:, h * cap + goff:h * cap + goff + gn]
                        nc.vector.tensor_scalar(dsl, ps[:, :gn],
                                                255.0, 0.0, A.min, A.max)
                        di += 1
                        goff2 += 6 * gn

                # stores: two pieces per slot (piece 1 only needs the first
                # three drains), alternating rings so the final receipts of
                # the last slots overlap across rings; tiny tail piece so
                # the exit-gating HBM write receipt covers little data
                ob = 2 * int(base[s])
                sg0 = _groups(cap)[0][1]
                cut1 = cap + sg0               # h0 full + h1 group 0
                ring = nc.scalar if s % 2 == 0 else nc.sync
                if s < NBLK - 1:
                    ring.dma_start(out[:, ob:ob + cut1], outt[:, :cut1])
                    ring.dma_start(out[:, ob + cut1:ob + 2 * cap],
                                   outt[:, cut1:2 * cap])
                else:
                    cut2 = 2 * cap - 64
                    nc.sync.dma_start(out[:, ob:ob + cut1], outt[:, :cut1])
                    nc.scalar.dma_start(out[:, ob + cut1:ob + cut2],
                                        outt[:, cut1:cut2])
                    nc.sync.dma_start(out[:, ob + cut2:ob + 2 * cap],
                                      outt[:, cut2:2 * cap])

    nc.compile()
    return nc


def _prep(values, lengths, kings):
    """Host prep: counts in merged-table column space, block assignment."""
    values = np.asarray(values).astype(np.int64)
    lengths = np.asarray(lengths).astype(np.int64)
    kings = np.asarray(kings).astype(np.int64)
    nb = lengths.shape[0]

    seg = np.repeat(np.arange(nb, dtype=np.int64), lengths)
    cnt_a = np.bincount(seg * PIECE + values,
                        minlength=nb * PIECE).reshape(nb, PIECE)
    cnt_b = np.bincount(seg * PIECE + _FLIP_COL[values],
                        minlength=nb * PIECE).reshape(nb, PIECE)
    cnt = np.concatenate([cnt_a, cnt_b])        # [2B, PIECE] ints

    blk = np.concatenate([kings[:, 0], _PERM[kings[:, 1]]])
    sizes = np.bincount(blk, minlength=64)
    order = np.argsort(blk, kind="stable")
    offs = np.concatenate([[0], np.cumsum(sizes)])

    rank = np.argsort(-sizes, kind="stable")    # block ids, desc size
    caps = tuple(int((sizes[rank[s * N_CORES]] + 7) // 8 * 8) or 8
                 for s in range(NBLK))
    base = np.concatenate([[0], np.cumsum(caps)]).astype(int)
    sum_caps = int(base[-1])

    pad_idx = np.full((N_CORES, sum_caps), -1, np.int64)
    for s in range(NBLK):
        for c in range(N_CORES):
            t = rank[s * N_CORES + c]
            ids = order[offs[t]:offs[t + 1]]
            pad_idx[c, base[s]:base[s] + len(ids)] = ids

    blk_of = rank.reshape(NBLK, N_CORES).T      # [core, slot]

    # rows where any clipped count would corrupt the result -> host recompute
    over_rows = np.unique(np.argwhere(cnt > 15)[:, 0])
    return cnt, pad_idx, caps, blk_of, over_rows


def _merged_tables(pieces, ranks, files, tiles, mask):
    p = np.asarray(pieces, np.float32)
    r = np.asarray(ranks, np.float32)
    f = np.asarray(files, np.float32)
    t = np.asarray(tiles, np.float32)
    m = np.asarray(mask, np.float32)
    return (t + (p + r + f) * m).reshape(64, PIECE, DOUT)


def _make_tab(merged16, blk_of):
    """Per-core [128, NBLK*TABW] fp16, tile (s, h*6+j)[p, d] =
    255 * merged[blk(c,s), j*128+p, h*128+d]."""
    planes = merged16.reshape(64, 6, 128, 2, 128)   # [blk, j, p, h, d]
    tabs = []
    for c in range(N_CORES):
        tc_ = planes[blk_of[c]]                    # [8, 6, 128, 2, 128]
        tabs.append(np.ascontiguousarray(
            tc_.transpose(2, 0, 3, 1, 4).reshape(128, -1)))
    return tabs


def _make_cm(cnt_ext, pad_idx, caps):
    """Per-core fp8 count planes [128, sum(6*cap)], group-major per slot."""
    base = np.concatenate([[0], np.cumsum(caps)]).astype(int)
    fp8 = ml_dtypes.float8_e4m3
    cms = []
    for c in range(N_CORES):
        parts = []
        for s in range(NBLK):
            cap = caps[s]
            for (goff, gn) in _groups(cap):
                ids = pad_idx[c, base[s] + goff:base[s] + goff + gn]
                sel = cnt_ext[ids]                     # [gn, 768] u8
                # [gn, 6, 128] -> [128(p), 6(j), gn(m)]
                t = sel.reshape(gn, 6, 128).transpose(2, 1, 0)
                parts.append(t.reshape(128, 6 * gn))
        cms.append(np.ascontiguousarray(
            np.concatenate(parts, axis=1)).astype(fp8))
    return cms


def _run(inputs, trace=False):
    cnt, pad_idx, caps, blk_of, over_rows = _prep(
        inputs["values"], inputs["lengths"], inputs["kings"])

    merged = _merged_tables(inputs["pieces"], inputs["ranks"],
                            inputs["files"], inputs["tiles"],
                            inputs["factorization_mask"])
    merged16 = (merged * OSCALE).astype(np.float16)

    key = caps
    if key not in _prog_cache:
        _prog_cache[key] = _build_program(caps)
    nc = _prog_cache[key]

    nb2 = cnt.shape[0]
    cnt_unclipped = cnt[over_rows].copy() if len(over_rows) else None
    np.minimum(cnt, 15, out=cnt, casting="unsafe")
    cnt_ext = np.zeros((nb2 + 1, PIECE), np.uint8)
    cnt_ext[:nb2] = cnt

    tabs = _make_tab(merged16, blk_of)
    cms = _make_cm(cnt_ext, pad_idx, caps)
    in_maps = [{"tab": tabs[c], "cm": cms[c]} for c in range(N_CORES)]

    res = run_bass_kernel_spmd(nc, in_maps, list(range(N_CORES)),
                               trace=trace)

    base = np.concatenate([[0], np.cumsum(caps)]).astype(int)
    comb = np.zeros((nb2, DOUT), np.float32)
    inv = np.float32(1.0 / OSCALE)
    for c in range(N_CORES):
        flat = res.results[c]["out"]               # [128, 2*sum_caps] u8
        for s in range(NBLK):
            cap = caps[s]
            # [128, 2, cap] -> [cap, 2, 128] -> [cap, 256]
            rows = (flat[:, 2 * base[s]:2 * (base[s] + cap)]
                    .astype(np.float32)
                    .reshape(128, 2, cap).transpose(2, 1, 0)
                    .reshape(cap, DOUT))
            ids = pad_idx[c, base[s]:base[s] + cap]
            valid = ids >= 0
            comb[ids[valid]] = rows[valid]
    comb *= inv
    np.clip(comb, 0.0, 1.0, out=comb)

    if len(over_rows):
        # counts were clipped at 15 on device for these rows: recompute f32
        kings = np.asarray(inputs["kings"]).astype(np.int64)
        row_blk = np.concatenate([kings[:, 0], _PERM[kings[:, 1]]])
        for i, r in enumerate(over_rows):
            comb[r] = np.clip(
                cnt_unclipped[i].astype(np.float32) @ merged[row_blk[r]],
                0.0, 1.0)

    return (comb[:B], comb[B:]), res


def kernel(**inputs):
    (a, b), _ = _run(inputs, trace=False)
    return a, b
